# revision 15
# baseline (speedup 1.0000x reference)
"""MinCutPool kernel for 8 Trainium2 NeuronCores (Bass/Tile, SPMD).

Math (reference):
    S   = softmax(x @ Wa + ba)               [N, K]
    x_t = x @ Wf + bf                        [N, D]
    A[r, c] = w  (scatter-set, duplicates overwrite), A := A + A^T
    out = (S^T A S) / (colsum(S)[:, None] + eps)
    pooled_x = S^T x_t
    pooled_edge_weight = out[off-diagonal, row-major]

Device strategy (per core, identical program, data-sharded):
  * Edges are deduped host-side (last-wins set semantics) and sharded
    8 x 32768.  S^T A S = sum_e w_e * S[r_e] (x) S[c_e]: gather S rows for
    the edge shard with dma_gather, scale by w, accumulate 128-edge
    matmuls into one PSUM tile -> M_partial [K, K].
  * Every core computes the full S (replicated) from a host-transposed,
    per-core *rotated* x^T, so its node shard is always rows 0:1024 of its
    own S -> identical program across cores, no collectives.
  * pooled_x uses associativity: S^T (x Wf + bf) = (S^T x) Wf + colsum (x) bf.
    Device computes P1_partial = S_shard^T @ x_shard; the [K,D]@[D,D] Wf
    epilogue, colsum, normalization and off-diagonal extraction are O(K*D^2)
    host epilogue work (~100x smaller than the device work).
"""

import numpy as np

N, D, K = 8192, 256, 64
E = 262144
NCORES = 8
ESH = E // NCORES          # edges per core (padded with zero-weight edges)
CHUNK = 2048               # edges per dma_gather call
NCHUNK = ESH // CHUNK
NT = N // 128              # node tiles (64)
SHT = (N // NCORES) // 128  # shard node tiles (8)
GRP = 4                    # node tiles per softmax group (one PSUM bank)
EPS = 1e-8

_CACHED_NC = None


def build_nc():
    import concourse.bacc as bacc
    import concourse.tile as tile
    from concourse import mybir
    from concourse.masks import make_identity

    nc = bacc.Bacc("TRN2", target_bir_lowering=False, debug=False,
                   num_devices=NCORES, num_swdge_queues=4)
    f32 = mybir.dt.float32

    f32r = mybir.dt.float32r
    xT = nc.dram_tensor("xT", [D, N], f32r, kind="ExternalInput")
    xsh = nc.dram_tensor("xsh", [N // NCORES, D], f32, kind="ExternalInput")
    Wa = nc.dram_tensor("Wa", [D, K], f32r, kind="ExternalInput")
    bab = nc.dram_tensor("bab", [128, GRP * K], f32, kind="ExternalInput")
    ridx = nc.dram_tensor("ridx", [128, ESH // 16], mybir.dt.int16,
                          kind="ExternalInput")
    cidx = nc.dram_tensor("cidx", [128, ESH // 16], mybir.dt.int16,
                          kind="ExternalInput")
    ew = nc.dram_tensor("ew", [128, ESH // 128], f32, kind="ExternalInput")

    S_blk = nc.dram_tensor("S_blk", [NT, 128, K], f32, kind="ExternalOutput")
    M_out = nc.dram_tensor("M_out", [K, K], f32, kind="ExternalOutput")
    P1_out = nc.dram_tensor("P1_out", [K, D], f32, kind="ExternalOutput")

    with tile.TileContext(nc) as tc:
        with tc.tile_pool(name="consts", bufs=1) as consts, \
             tc.tile_pool(name="xtp", bufs=4) as xtp, \
             tc.tile_pool(name="sbig", bufs=1) as sbig, \
             tc.tile_pool(name="gat", bufs=6) as gat, \
             tc.tile_pool(name="misc", bufs=2) as misc, \
             tc.tile_pool(name="ps", bufs=3, space="PSUM") as ps, \
             tc.tile_pool(name="psacc", bufs=1, space="PSUM") as psacc:

            # ---- constants ----
            wa_t = consts.tile([128, 2, K], f32r)
            nc.scalar.dma_start(out=wa_t[:],
                                in_=Wa.ap().rearrange("(b p) k -> p b k", p=128))
            bab_t = consts.tile([128, GRP * K], f32)
            nc.scalar.dma_start(out=bab_t[:], in_=bab[:])
            ridx_t = consts.tile([128, ESH // 16], mybir.dt.int16)
            nc.scalar.dma_start(out=ridx_t[:], in_=ridx[:])
            cidx_t = consts.tile([128, ESH // 16], mybir.dt.int16)
            nc.scalar.dma_start(out=cidx_t[:], in_=cidx[:])
            ew_t = consts.tile([128, ESH // 128], f32)
            nc.scalar.dma_start(out=ew_t[:], in_=ew[:])

            # ---- S = softmax(x @ Wa + ba), all N nodes ----
            s_sb = sbig.tile([128, NT, K], f32)  # 2MB, node tile i at [:, i, :]
            xT_r = xT.ap().rearrange("(b p) n -> p b n", p=128)
            n_groups = NT // GRP
            for g in range(n_groups):
                # load x^T columns for this group of GRP node tiles
                xt_c = xtp.tile([128, 2, GRP * 128], f32r, tag="xt")
                eng = nc.sync if g % 2 == 0 else nc.scalar
                eng.dma_start(
                    out=xt_c[:],
                    in_=xT_r[:, :, g * GRP * 128:(g + 1) * GRP * 128])
                spre = ps.tile([128, GRP, K], f32, space="PSUM", tag="spre")
                for t in range(GRP):
                    for b in range(2):
                        nc.tensor.matmul(
                            spre[:, t, :],
                            lhsT=xt_c[:, b, t * 128:(t + 1) * 128],
                            rhs=wa_t[:, b, :],
                            start=(b == 0), stop=(b == 1))
                sl = s_sb[:, g * GRP:(g + 1) * GRP, :]
                # +ba (PSUM -> SBUF), exp, row-sum, normalize
                nc.vector.tensor_tensor(
                    out=sl, in0=spre[:],
                    in1=bab_t[:].rearrange("p (t k) -> p t k", t=GRP),
                    op=mybir.AluOpType.add)
                nc.scalar.activation(out=sl, in_=sl,
                                     func=mybir.ActivationFunctionType.Exp)
                ssum = misc.tile([128, GRP], f32, tag="ssum")
                nc.vector.reduce_sum(out=ssum[:], in_=sl,
                                     axis=mybir.AxisListType.X)
                nc.vector.reciprocal(out=ssum[:], in_=ssum[:])
                nc.vector.tensor_tensor(
                    out=sl, in0=sl,
                    in1=ssum[:].unsqueeze(2).to_broadcast([128, GRP, K]),
                    op=mybir.AluOpType.mult)

            # ---- store S (i-major: flat row == node index), halves ----
            S_r = S_blk.ap().rearrange("i p k -> p i k")
            nc.sync.dma_start(out=S_r[:, :NT // 4, :],
                              in_=s_sb[:, :NT // 4, :])
            nc.scalar.dma_start(out=S_r[:, NT // 4:NT // 2, :],
                                in_=s_sb[:, NT // 4:NT // 2, :])
            nc.sync.dma_start(out=S_r[:, NT // 2:3 * NT // 4, :],
                              in_=s_sb[:, NT // 2:3 * NT // 4, :])
            nc.scalar.dma_start(out=S_r[:, 3 * NT // 4:, :],
                                in_=s_sb[:, 3 * NT // 4:, :])
            s_flat = S_blk.ap().rearrange("i p k -> (i p) k")
            s_half = s_flat[:N // 2, :]   # dep on first half-store only

            # ---- edge shard: M_partial = sum_e w_e S[r_e] (x) S[c_e] ----
            m_ps = psacc.tile([K, K], f32, space="PSUM", tag="macc")
            for c in range(NCHUNK):
                sr = gat.tile([128, CHUNK // 128, K], f32, tag="sr")
                sc = gat.tile([128, CHUNK // 128, K], f32, tag="sc")
                isl = slice(c * CHUNK // 16, (c + 1) * CHUNK // 16)
                src_ap = s_half if c < 3 else s_flat
                nc.gpsimd.dma_gather(
                    out_ap=sr[:], in_ap=src_ap, idxs_ap=ridx_t[:, isl],
                    num_idxs=CHUNK, num_idxs_reg=CHUNK, elem_size=K,
                    single_packet=False, queue_num=(2 * c) % 4)
                nc.gpsimd.dma_gather(
                    out_ap=sc[:], in_ap=src_ap, idxs_ap=cidx_t[:, isl],
                    num_idxs=CHUNK, num_idxs_reg=CHUNK, elem_size=K,
                    single_packet=False, queue_num=(2 * c + 1) % 4)
                nblk = CHUNK // 128
                bf16 = mybir.dt.bfloat16
                wsl = ew_t[:, c * nblk:(c + 1) * nblk]
                srb = gat.tile([128, nblk, K], bf16, tag="srb")
                nc.vector.tensor_copy(out=srb[:], in_=sr[:])
                scb = gat.tile([128, nblk, K], bf16, tag="scb")
                nc.vector.tensor_tensor(
                    out=scb[:], in0=sc[:],
                    in1=wsl.unsqueeze(2).to_broadcast([128, nblk, K]),
                    op=mybir.AluOpType.mult)
                for b in range(nblk):
                    nc.tensor.matmul(
                        m_ps[:], lhsT=srb[:, b, :], rhs=scb[:, b, :],
                        start=(c == 0 and b == 0),
                        stop=(c == NCHUNK - 1 and b == nblk - 1))
            m_sb = misc.tile([K, K], f32, tag="mout")
            nc.vector.tensor_copy(out=m_sb[:], in_=m_ps[:])
            nc.sync.dma_start(out=M_out[:], in_=m_sb[:])

            # ---- P1_partial = S_shard^T @ x_shard ----
            xsh_t = sbig.tile([128, SHT, D], f32)
            nc.sync.dma_start(
                out=xsh_t[:],
                in_=xsh.ap().rearrange("(i p) d -> p i d", p=128))
            p1_ps = psacc.tile([K, D], f32, space="PSUM", tag="p1")
            for i in range(SHT):
                nc.tensor.matmul(p1_ps[:], lhsT=s_sb[:, i, :],
                                 rhs=xsh_t[:, i, :],
                                 start=(i == 0), stop=(i == SHT - 1))
            p1_sb = misc.tile([K, D], f32, tag="p1out")
            nc.vector.tensor_copy(out=p1_sb[:], in_=p1_ps[:])
            nc.sync.dma_start(out=P1_out[:], in_=p1_sb[:])


    nc.compile()
    return nc


def get_nc():
    global _CACHED_NC
    if _CACHED_NC is None:
        _CACHED_NC = build_nc()
    return _CACHED_NC


def _wrap_idx(ii):
    """[ESH] int -> [128, ESH//16] int16 (16-partition wrap, replicated x8)."""
    a = np.ascontiguousarray(ii.reshape(ESH // 16, 16).T.astype(np.int16))
    return np.tile(a, (8, 1))


def prepare_inputs(x, edge_index, edge_weight, Wa, ba):
    x = np.asarray(x, dtype=np.float32)
    ei = np.asarray(edge_index)
    w = np.asarray(edge_weight, dtype=np.float32)
    Wa = np.asarray(Wa, dtype=np.float32)
    ba = np.asarray(ba, dtype=np.float32)

    # dedup, last occurrence wins (matches scatter-set semantics)
    row = ei[0].astype(np.int64)
    col = ei[1].astype(np.int64)
    key = row * N + col
    _, first_rev = np.unique(key[::-1], return_index=True)
    keep = key.shape[0] - 1 - first_rev
    r_u, c_u, w_u = row[keep], col[keep], w[keep]
    eu = r_u.shape[0]
    r_p = np.zeros(E, dtype=np.int64)
    c_p = np.zeros(E, dtype=np.int64)
    w_p = np.zeros(E, dtype=np.float32)
    r_p[:eu], c_p[:eu], w_p[:eu] = r_u, c_u, w_u

    xT = np.ascontiguousarray(x.T)  # [D, N]
    bab = np.broadcast_to(np.tile(ba, GRP), (128, GRP * K)).copy()

    in_maps = []
    for c in range(NCORES):
        start = (N // NCORES) * c
        xT_rot = (xT if start == 0 else
                  np.ascontiguousarray(
                      np.concatenate([xT[:, start:], xT[:, :start]], axis=1)))
        sl = slice(c * ESH, (c + 1) * ESH)
        rr = (r_p[sl] - start) % N
        cc = (c_p[sl] - start) % N
        ww = w_p[sl]
        # bucket: first 3*CHUNK slots must have both endpoints < N/2 (these
        # chunks gather right after the first half-store). ~N/4 of edges
        # qualify (~8192 >> 6144), so this always fills.
        early = (rr < N // 2) & (cc < N // 2)
        order = np.argsort(~early, kind="stable")
        nearly = int(early.sum())
        assert nearly >= 3 * CHUNK, nearly
        rr, cc, ww = rr[order], cc[order], ww[order]
        in_maps.append({
            "xT": xT_rot,
            "xsh": np.ascontiguousarray(x[start:start + N // NCORES]),
            "Wa": Wa,
            "bab": bab,
            "ridx": _wrap_idx(rr),
            "cidx": _wrap_idx(cc),
            "ew": np.ascontiguousarray(
                ww.reshape(ESH // 128, 128).T),
        })
    return in_maps


def combine_outputs(results, Wf, bf, index_dtype):
    Wf = np.asarray(Wf, dtype=np.float32)
    bf = np.asarray(bf, dtype=np.float32)
    s_blk = results[0]["S_blk"]                       # core 0: rotation = 0
    S = np.ascontiguousarray(s_blk.reshape(N, K))     # i-major: row == node
    M = np.zeros((K, K), dtype=np.float64)
    P1 = np.zeros((K, D), dtype=np.float64)
    for r in results:
        M += r["M_out"].astype(np.float64)
        P1 += r["P1_out"].astype(np.float64)
    colsum = S.astype(np.float64).sum(axis=0)          # [K]
    out = (M + M.T) / (colsum[:, None] + EPS)
    pooled_x = (P1 @ Wf.astype(np.float64)
                + colsum[:, None] * bf.astype(np.float64)[None, :])
    ii, jj = np.nonzero(~np.eye(K, dtype=bool))
    pooled_edge_index = np.stack([ii, jj]).astype(index_dtype)
    pooled_edge_weight = out[ii, jj].astype(np.float32)
    return (pooled_x.astype(np.float32), pooled_edge_index,
            pooled_edge_weight, S)


def kernel(x, edge_index, edge_weight, Wa, ba, Wf, bf):
    from concourse.bass_utils import run_bass_kernel_spmd
    nc = get_nc()
    in_maps = prepare_inputs(x, edge_index, edge_weight, Wa, ba)
    res = run_bass_kernel_spmd(nc, in_maps, core_ids=list(range(NCORES)))
    return combine_outputs(res.results, Wf, bf,
                           np.asarray(edge_index).dtype)


# revision 17
# speedup vs baseline: 1.0854x; 1.0854x over previous
"""MinCutPool kernel for 8 Trainium2 NeuronCores (Bass/Tile, SPMD).

Math (reference):
    S   = softmax(x @ Wa + ba)               [N, K]
    x_t = x @ Wf + bf                        [N, D]
    A[r, c] = w  (scatter-set, duplicates overwrite), A := A + A^T
    out = (S^T A S) / (colsum(S)[:, None] + eps)
    pooled_x = S^T x_t
    pooled_edge_weight = out[off-diagonal, row-major]

Device strategy (per core, identical program, data-sharded):
  * Edges are deduped host-side (last-wins set semantics) and sharded
    8 x 32768.  S^T A S = sum_e w_e * S[r_e] (x) S[c_e]: gather S rows for
    the edge shard with dma_gather, scale by w, accumulate 128-edge
    matmuls into one PSUM tile -> M_partial [K, K].
  * Every core computes the full S (replicated) from a host-transposed,
    per-core *rotated* x^T, so its node shard is always rows 0:1024 of its
    own S -> identical program across cores, no collectives.
  * pooled_x uses associativity: S^T (x Wf + bf) = (S^T x) Wf + colsum (x) bf.
    Device computes P1_partial = S_shard^T @ x_shard; the [K,D]@[D,D] Wf
    epilogue, colsum, normalization and off-diagonal extraction are O(K*D^2)
    host epilogue work (~100x smaller than the device work).
"""

import numpy as np

N, D, K = 8192, 256, 64
E = 262144
NCORES = 8
ESH = E // NCORES          # edges per core (padded with zero-weight edges)
CHUNK = 2048               # edges per dma_gather call
NCHUNK = ESH // CHUNK
NT = N // 128              # node tiles (64)
SHT = (N // NCORES) // 128  # shard node tiles (8)
GRP = 4                    # node tiles per softmax group (one PSUM bank)
EPS = 1e-8

_CACHED_NC = None


def build_nc():
    import concourse.bacc as bacc
    import concourse.tile as tile
    from concourse import mybir
    from concourse.masks import make_identity

    nc = bacc.Bacc("TRN2", target_bir_lowering=False, debug=False,
                   num_devices=NCORES, num_swdge_queues=4)
    f32 = mybir.dt.float32

    f32r = mybir.dt.float32r
    xT = nc.dram_tensor("xT", [D, N], f32r, kind="ExternalInput")
    xsh = nc.dram_tensor("xsh", [N // NCORES, D], f32, kind="ExternalInput")
    Wa = nc.dram_tensor("Wa", [D, K], f32r, kind="ExternalInput")
    bab = nc.dram_tensor("bab", [128, GRP * K], f32, kind="ExternalInput")
    ridx = nc.dram_tensor("ridx", [128, ESH // 16], mybir.dt.int16,
                          kind="ExternalInput")
    cidx = nc.dram_tensor("cidx", [128, ESH // 16], mybir.dt.int16,
                          kind="ExternalInput")
    ew = nc.dram_tensor("ew", [128, ESH // 128], f32, kind="ExternalInput")

    S_blk = nc.dram_tensor("S_blk", [NT, 128, K], f32, kind="ExternalOutput")
    M_out = nc.dram_tensor("M_out", [K, K], f32, kind="ExternalOutput")
    P1_out = nc.dram_tensor("P1_out", [K, D], f32, kind="ExternalOutput")

    with tile.TileContext(nc) as tc:
        with tc.tile_pool(name="consts", bufs=1) as consts, \
             tc.tile_pool(name="xtp", bufs=4) as xtp, \
             tc.tile_pool(name="sbig", bufs=1) as sbig, \
             tc.tile_pool(name="gat", bufs=6) as gat, \
             tc.tile_pool(name="misc", bufs=2) as misc, \
             tc.tile_pool(name="ps", bufs=3, space="PSUM") as ps, \
             tc.tile_pool(name="psacc", bufs=1, space="PSUM") as psacc:

            # ---- constants ----
            wa_t = consts.tile([128, 2, K], f32r)
            nc.scalar.dma_start(out=wa_t[:],
                                in_=Wa.ap().rearrange("(b p) k -> p b k", p=128))
            bab_t = consts.tile([128, GRP * K], f32)
            nc.scalar.dma_start(out=bab_t[:], in_=bab[:])
            ridx_t = consts.tile([128, ESH // 16], mybir.dt.int16)
            nc.scalar.dma_start(out=ridx_t[:], in_=ridx[:])
            cidx_t = consts.tile([128, ESH // 16], mybir.dt.int16)
            nc.scalar.dma_start(out=cidx_t[:], in_=cidx[:])
            ew_t = consts.tile([128, ESH // 128], f32)
            nc.scalar.dma_start(out=ew_t[:], in_=ew[:])

            # ---- S = softmax(x @ Wa + ba), all N nodes ----
            s_sb = sbig.tile([128, NT, K], f32)  # 2MB, node tile i at [:, i, :]
            xT_r = xT.ap().rearrange("(b p) n -> p b n", p=128)
            n_groups = NT // GRP
            for g in range(n_groups):
                # load x^T columns for this group of GRP node tiles
                xt_c = xtp.tile([128, 2, GRP * 128], f32r, tag="xt")
                eng = nc.sync if g % 2 == 0 else nc.scalar
                eng.dma_start(
                    out=xt_c[:],
                    in_=xT_r[:, :, g * GRP * 128:(g + 1) * GRP * 128])
                spre = ps.tile([128, GRP, K], f32, space="PSUM", tag="spre")
                for t in range(GRP):
                    for b in range(2):
                        nc.tensor.matmul(
                            spre[:, t, :],
                            lhsT=xt_c[:, b, t * 128:(t + 1) * 128],
                            rhs=wa_t[:, b, :],
                            start=(b == 0), stop=(b == 1))
                sl = s_sb[:, g * GRP:(g + 1) * GRP, :]
                # +ba (PSUM -> SBUF), exp, row-sum, normalize
                nc.vector.tensor_tensor(
                    out=sl, in0=spre[:],
                    in1=bab_t[:].rearrange("p (t k) -> p t k", t=GRP),
                    op=mybir.AluOpType.add)
                nc.scalar.activation(out=sl, in_=sl,
                                     func=mybir.ActivationFunctionType.Exp)
                ssum = misc.tile([128, GRP], f32, tag="ssum")
                nc.vector.reduce_sum(out=ssum[:], in_=sl,
                                     axis=mybir.AxisListType.X)
                nc.vector.reciprocal(out=ssum[:], in_=ssum[:])
                nc.vector.tensor_tensor(
                    out=sl, in0=sl,
                    in1=ssum[:].unsqueeze(2).to_broadcast([128, GRP, K]),
                    op=mybir.AluOpType.mult)

            # ---- store S (i-major: flat row == node index), quarters ----
            S_r = S_blk.ap().rearrange("i p k -> p i k")
            for q in range(4):
                seng = nc.sync if q % 2 == 0 else nc.scalar
                seng.dma_start(
                    out=S_r[:, q * NT // 4:(q + 1) * NT // 4, :],
                    in_=s_sb[:, q * NT // 4:(q + 1) * NT // 4, :])
            s_flat = S_blk.ap().rearrange("i p k -> (i p) k")
            s_b0 = s_flat[:N // 2, :]      # ready after quarter stores 0,1
            s_b1 = s_flat[:3 * N // 4, :]  # ready after quarter stores 0,1,2

            # ---- edge shard: M_partial = sum_e w_e S[r_e] (x) S[c_e] ----
            m_ps = psacc.tile([K, K], f32, space="PSUM", tag="macc")
            for c in range(NCHUNK):
                sr = gat.tile([128, CHUNK // 128, K], f32, tag="sr")
                sc = gat.tile([128, CHUNK // 128, K], f32, tag="sc")
                isl = slice(c * CHUNK // 16, (c + 1) * CHUNK // 16)
                src_ap = s_b0 if c < 3 else (s_b1 if c < 8 else s_flat)
                nc.gpsimd.dma_gather(
                    out_ap=sr[:], in_ap=src_ap, idxs_ap=ridx_t[:, isl],
                    num_idxs=CHUNK, num_idxs_reg=CHUNK, elem_size=K,
                    single_packet=False, queue_num=(2 * c) % 4)
                nc.gpsimd.dma_gather(
                    out_ap=sc[:], in_ap=src_ap, idxs_ap=cidx_t[:, isl],
                    num_idxs=CHUNK, num_idxs_reg=CHUNK, elem_size=K,
                    single_packet=False, queue_num=(2 * c + 1) % 4)
                nblk = CHUNK // 128
                bf16 = mybir.dt.bfloat16
                wsl = ew_t[:, c * nblk:(c + 1) * nblk]
                srb = gat.tile([128, nblk, K], bf16, tag="srb")
                nc.vector.tensor_copy(out=srb[:], in_=sr[:])
                scb = gat.tile([128, nblk, K], bf16, tag="scb")
                nc.vector.tensor_tensor(
                    out=scb[:], in0=sc[:],
                    in1=wsl.unsqueeze(2).to_broadcast([128, nblk, K]),
                    op=mybir.AluOpType.mult)
                for b in range(nblk):
                    nc.tensor.matmul(
                        m_ps[:], lhsT=srb[:, b, :], rhs=scb[:, b, :],
                        start=(c == 0 and b == 0),
                        stop=(c == NCHUNK - 1 and b == nblk - 1))
            m_sb = misc.tile([K, K], f32, tag="mout")
            nc.vector.tensor_copy(out=m_sb[:], in_=m_ps[:])
            nc.sync.dma_start(out=M_out[:], in_=m_sb[:])

            # ---- P1_partial = S_shard^T @ x_shard ----
            xsh_t = sbig.tile([128, SHT, D], f32)
            nc.sync.dma_start(
                out=xsh_t[:],
                in_=xsh.ap().rearrange("(i p) d -> p i d", p=128))
            p1_ps = psacc.tile([K, D], f32, space="PSUM", tag="p1")
            for i in range(SHT):
                nc.tensor.matmul(p1_ps[:], lhsT=s_sb[:, i, :],
                                 rhs=xsh_t[:, i, :],
                                 start=(i == 0), stop=(i == SHT - 1))
            p1_sb = misc.tile([K, D], f32, tag="p1out")
            nc.vector.tensor_copy(out=p1_sb[:], in_=p1_ps[:])
            nc.sync.dma_start(out=P1_out[:], in_=p1_sb[:])


    nc.compile()
    return nc


def get_nc():
    global _CACHED_NC
    if _CACHED_NC is None:
        _CACHED_NC = build_nc()
    return _CACHED_NC


def _wrap_idx(ii):
    """[ESH] int -> [128, ESH//16] int16 (16-partition wrap, replicated x8)."""
    a = np.ascontiguousarray(ii.reshape(ESH // 16, 16).T.astype(np.int16))
    return np.tile(a, (8, 1))


def prepare_inputs(x, edge_index, edge_weight, Wa, ba):
    x = np.asarray(x, dtype=np.float32)
    ei = np.asarray(edge_index)
    w = np.asarray(edge_weight, dtype=np.float32)
    Wa = np.asarray(Wa, dtype=np.float32)
    ba = np.asarray(ba, dtype=np.float32)

    # dedup, last occurrence wins (matches scatter-set semantics)
    row = ei[0].astype(np.int64)
    col = ei[1].astype(np.int64)
    key = row * N + col
    _, first_rev = np.unique(key[::-1], return_index=True)
    keep = key.shape[0] - 1 - first_rev
    r_u, c_u, w_u = row[keep], col[keep], w[keep]
    eu = r_u.shape[0]
    r_p = np.zeros(E, dtype=np.int64)
    c_p = np.zeros(E, dtype=np.int64)
    w_p = np.zeros(E, dtype=np.float32)
    r_p[:eu], c_p[:eu], w_p[:eu] = r_u, c_u, w_u

    xT = np.ascontiguousarray(x.T)  # [D, N]
    bab = np.broadcast_to(np.tile(ba, GRP), (128, GRP * K)).copy()

    in_maps = []
    for c in range(NCORES):
        start = (N // NCORES) * c
        xT_rot = (xT if start == 0 else
                  np.ascontiguousarray(
                      np.concatenate([xT[:, start:], xT[:, :start]], axis=1)))
        sl = slice(c * ESH, (c + 1) * ESH)
        rr = (r_p[sl] - start) % N
        cc = (c_p[sl] - start) % N
        ww = w_p[sl]
        # bucket: first 3*CHUNK slots must have both endpoints < N/2 (these
        # chunks gather right after the first half-store). ~N/4 of edges
        # qualify (~8192 >> 6144), so this always fills.
        e0 = (rr < N // 2) & (cc < N // 2)
        e1 = (rr < 3 * N // 4) & (cc < 3 * N // 4)
        tier = np.where(e0, 0, np.where(e1, 1, 2))
        order = np.argsort(tier, kind="stable")
        assert int(e0.sum()) >= 3 * CHUNK and int(e1.sum()) >= 8 * CHUNK
        rr, cc, ww = rr[order], cc[order], ww[order]
        in_maps.append({
            "xT": xT_rot,
            "xsh": np.ascontiguousarray(x[start:start + N // NCORES]),
            "Wa": Wa,
            "bab": bab,
            "ridx": _wrap_idx(rr),
            "cidx": _wrap_idx(cc),
            "ew": np.ascontiguousarray(
                ww.reshape(ESH // 128, 128).T),
        })
    return in_maps


def combine_outputs(results, Wf, bf, index_dtype):
    Wf = np.asarray(Wf, dtype=np.float32)
    bf = np.asarray(bf, dtype=np.float32)
    s_blk = results[0]["S_blk"]                       # core 0: rotation = 0
    S = np.ascontiguousarray(s_blk.reshape(N, K))     # i-major: row == node
    M = np.zeros((K, K), dtype=np.float64)
    P1 = np.zeros((K, D), dtype=np.float64)
    for r in results:
        M += r["M_out"].astype(np.float64)
        P1 += r["P1_out"].astype(np.float64)
    colsum = S.astype(np.float64).sum(axis=0)          # [K]
    out = (M + M.T) / (colsum[:, None] + EPS)
    pooled_x = (P1 @ Wf.astype(np.float64)
                + colsum[:, None] * bf.astype(np.float64)[None, :])
    ii, jj = np.nonzero(~np.eye(K, dtype=bool))
    pooled_edge_index = np.stack([ii, jj]).astype(index_dtype)
    pooled_edge_weight = out[ii, jj].astype(np.float32)
    return (pooled_x.astype(np.float32), pooled_edge_index,
            pooled_edge_weight, S)


def kernel(x, edge_index, edge_weight, Wa, ba, Wf, bf):
    from concourse.bass_utils import run_bass_kernel_spmd
    nc = get_nc()
    in_maps = prepare_inputs(x, edge_index, edge_weight, Wa, ba)
    res = run_bass_kernel_spmd(nc, in_maps, core_ids=list(range(NCORES)))
    return combine_outputs(res.results, Wf, bf,
                           np.asarray(edge_index).dtype)
